# revision 24
# baseline (speedup 1.0000x reference)
"""Fused ConvBNReLU1D (kernel_size=1) + per-tensor po2 weight/bias fake-quant
+ QuantReLU(8-bit unsigned) output fake-quant, on 8 Trainium2 NeuronCores.

Strategy
--------
- Host: quantize W/b (per-tensor po2 scales, depends only on W/b - "precomputed
  scale" option from the sharding hint).
- Device (SPMD, data-parallel over batch B=32 -> 4 batches/core):
  Phase A: pointwise GEMM y = relu(Wq @ x + bq) with float32r matmuls
           (fp32 operands truncated to FP22 in the PE; 1 cycle/row for
           free-dim >= 256). Weights DMA'd in 4 k-slices so the first
           matmul starts ~1.5us in. y stays resident in SBUF; per-chunk
           running maxes tracked on the vector engine.
  - AllGather of the per-partition max vectors (cheaper than AllReduce in
    both the HW and the cost model: no reduction pass), then each core
    reduces the gathered 8x128 values to the same global max locally.
  Phase B: out = round(y*inv)*s elementwise with the +/-1.5*2^23 magic
           round-to-nearest-even trick, split across the Activation engine
           (5 chunks, both passes) and the DVE (11 chunks, both passes) so
           neither engine exceeds the output-DMA time; outputs are written
           bf16 (quantized values span 8 bits, so bf16's 8+1 mantissa bits
           keep the error ~0.2% of absmax) and widened to fp32 on host.
"""

import os
import sys
from contextlib import ExitStack

import numpy as np

for _p in ("/opt/trn_rl_repo", os.path.expanduser("~/.axon_site/_ro/trn_rl_repo")):
    if os.path.isdir(_p) and _p not in sys.path:
        sys.path.insert(0, _p)

import concourse.bacc as bacc
import concourse.mybir as mybir
import concourse.tile as tile
from concourse.bass_utils import run_bass_kernel_spmd

P = 128
B, CIN, COUT, N = 32, 512, 512, 2048
NCORES = 8
BSH = B // NCORES          # batches per core
NT = 512                   # matmul free dim (= one PSUM bank of fp32)
KT = CIN // P              # 4 contraction tiles
MT = COUT // P             # 4 output-row tiles
NJ = N // NT               # 4 n-windows per batch
NCH = BSH * NJ             # 16 (batch, n-window) chunks per core
CH2 = MT * NT              # columns of y per chunk (2048)
NACT = 6                   # phase-B chunks processed on the Activation engine
MAGIC = 12582912.0         # 1.5 * 2^23: RNE rounding for t in [0, 2^22)
QMAX_S = 127.0
QMAX_U = 255.0

_cache = {}
LAST_RESULT = None         # BassKernelResults of the most recent run (test.py)


def _build():
    f32 = mybir.dt.float32
    f32r = mybir.dt.float32r
    bf16 = mybir.dt.bfloat16
    Relu = mybir.ActivationFunctionType.Relu
    Copy = mybir.ActivationFunctionType.Copy
    X = mybir.AxisListType.X
    Alu = mybir.AluOpType

    nc = bacc.Bacc(
        "TRN2",
        target_bir_lowering=False,
        debug=False,
        enable_asserts=False,
        num_devices=NCORES,
    )
    xs = nc.dram_tensor("xs", [BSH, CIN, N], f32r, kind="ExternalInput")
    wT = nc.dram_tensor("wT", [CIN, COUT], f32r, kind="ExternalInput")
    bqv = nc.dram_tensor("bqv", [P, MT], f32, kind="ExternalInput")
    out = nc.dram_tensor("out", [BSH, COUT, N], bf16, kind="ExternalOutput")

    with tile.TileContext(nc) as tc, ExitStack() as ctx:
        const = ctx.enter_context(tc.tile_pool(name="const", bufs=1))
        ypool = ctx.enter_context(tc.tile_pool(name="yp", bufs=1))
        pspool = ctx.enter_context(tc.tile_pool(name="ps", bufs=8, space="PSUM"))
        dram = ctx.enter_context(tc.tile_pool(name="dram", bufs=1, space="DRAM"))

        # Weights: lhsT tile (k, m) = Wq.T[k*128:(k+1)*128, m*128:(m+1)*128],
        # packed at column (k*MT+m)*P.  One DMA per k-slice (256 KiB) so the
        # first matmuls only wait on their own slice.
        wq = const.tile([P, KT * MT * P], f32r)

        def load_w_slice(k):
            nc.sync.dma_start(
                out=wq[:, k * MT * P:(k + 1) * MT * P].rearrange(
                    "p (m q) -> p m q", m=MT
                ),
                in_=wT[k * P:(k + 1) * P, :].rearrange("p (m q) -> p m q", q=P),
            )

        ybig = ypool.tile([P, NCH * CH2], f32)
        # per-m rows of UNBIASED psum maxes: max_n relu(ps+b) == relu(max_n
        # ps + b), so reductions run straight off PSUM (in parallel with the
        # Activation engine's relu+bias store) and the bias fixup happens once
        # on a [P, MT] tile at the very end.
        NCHF = NCH - 2            # chunks with full-width m-tiles
        maxb = const.tile([P, MT * NCHF], f32)
        lastm = const.tile([P, 2 * 2 * MT], f32)   # half-tile maxes, chunks 14/15
        rm = const.tile([P, MT], f32)
        rm2 = const.tile([P, MT], f32)
        mloc = const.tile([P, 1], f32)
        bias = const.tile([P, MT], f32)

        # x tiles live only for phase A; scoping the pool returns its SBUF to
        # the allocator for the phase-B double buffers
        with tc.tile_pool(name="xp", bufs=2) as xpool:

            def load_x_chunk(c):
                bb, j = divmod(c, NJ)
                xt = xpool.tile([P, KT * NT], f32r)
                for k in range(KT):
                    nc.sync.dma_start(
                        out=xt[:, k * NT:(k + 1) * NT],
                        in_=xs[bb, k * P:(k + 1) * P, j * NT:(j + 1) * NT],
                    )
                return xt

            # w_k0 first (smallest wait for matmul 0), then chunk-0 x slices,
            # then the rest of the weights
            load_w_slice(0)
            xtiles = {0: load_x_chunk(0)}
            for k in range(1, KT):
                load_w_slice(k)
            nc.sync.dma_start(out=bias[:], in_=bqv[:, :])

            # ---- Phase A: y = relu(Wq @ x + bq) with per-(m, chunk) maxes
            NT2 = NT // 2

            def fold_pairs(base):
                # max over the (h0, h1) half-tile pairs of one late chunk
                lv = lastm[:, base:base + 2 * MT].rearrange("p (m h) -> p h m", h=2)
                nc.vector.tensor_max(
                    rm2[:],
                    lv[:, 0:1, :].rearrange("p a m -> p (a m)"),
                    lv[:, 1:2, :].rearrange("p a m -> p (a m)"),
                )
                nc.vector.tensor_max(rm[:], rm[:], rm2[:])

            for c in range(NCH):
                xt = xtiles.pop(c) if c in xtiles else load_x_chunk(c)
                if c < NCHF:
                    for m in range(MT):
                        ps = pspool.tile([P, NT], f32)
                        for k in range(KT):
                            nc.tensor.matmul(
                                ps[:],
                                wq[:, (k * MT + m) * P:(k * MT + m + 1) * P],
                                xt[:, k * NT:(k + 1) * NT],
                                start=(k == 0),
                                stop=(k == KT - 1),
                            )
                        col = (c * MT + m) * NT
                        nc.scalar.activation(
                            ybig[:, col:col + NT], ps[:], Relu, bias=bias[:, m:m + 1]
                        )
                        nc.vector.reduce_max(
                            maxb[:, m * NCHF + c:m * NCHF + c + 1], ps[:], axis=X
                        )
                else:
                    if c == NCH - 1:
                        # fold chunk 14's half-tile pairs while its reduces
                        # are fresh (hidden under chunk 15's matmuls)
                        fold_pairs(0)
                    # the last two chunks run half-width m-tiles so the DVE
                    # reduce train keeps pace with the matmuls and the final
                    # exposed reduce covers only 256 columns
                    base = (c - NCHF) * 2 * MT
                    for m in range(MT):
                        for h in range(2):
                            ps = pspool.tile([P, NT2], f32)
                            for k in range(KT):
                                nc.tensor.matmul(
                                    ps[:],
                                    wq[:, (k * MT + m) * P:(k * MT + m + 1) * P],
                                    xt[:, k * NT + h * NT2:k * NT + h * NT2 + NT2],
                                    start=(k == 0),
                                    stop=(k == KT - 1),
                                )
                            col = (c * MT + m) * NT + h * NT2
                            nc.scalar.activation(
                                ybig[:, col:col + NT2], ps[:], Relu,
                                bias=bias[:, m:m + 1],
                            )
                            nc.vector.reduce_max(
                                lastm[:, base + m * 2 + h:base + m * 2 + h + 1],
                                ps[:], axis=X,
                            )
                if c == NCHF - 1:
                    # pre-reduce the full-width chunks per m while the
                    # half-tile chunks compute
                    for mm in range(MT):
                        nc.vector.reduce_max(
                            rm[:, mm:mm + 1],
                            maxb[:, mm * NCHF:(mm + 1) * NCHF],
                            axis=X,
                        )

        # exposed tail: fold chunk 15's half-tile pairs, add the bias once
        # per m, reduce to the local max
        fold_pairs(2 * MT)
        nc.vector.tensor_add(rm[:], rm[:], bias[:])
        nc.vector.reduce_max(mloc[:], rm[:], axis=X)
        cc_in = dram.tile([1, P], f32)
        cc_out = dram.tile([1, NCORES * P], f32)
        nc.sync.dma_start(out=cc_in[:].rearrange("a b -> b a"), in_=mloc[:])
        nc.gpsimd.collective_compute(
            "AllGather",
            Alu.bypass,
            replica_groups=[list(range(NCORES))],
            ins=[cc_in.opt()],
            outs=[cc_out.opt()],
        )
        # transposed gather load: partition p reads rank r's max of partition p
        # at flat index r*128+p, so every partition sees all 8 ranks' values
        # and computes the (identical) scale locally - no broadcast needed.
        grow = const.tile([P, NCORES], f32)
        nc.sync.dma_start(
            out=grow[:],
            in_=cc_out[:].rearrange("a (r p) -> p (a r)", p=P),
        )

        # per-partition scale chain: scal = [inv, s, -MAGIC*s],
        # inv = 255/gmax refined with one Newton step on reciprocal.
        scal = const.tile([P, 4], f32)
        gmax = const.tile([P, 1], f32)
        i0 = const.tile([P, 1], f32)
        e = const.tile([P, 1], f32)
        nc.vector.reduce_max(gmax[:], grow[:], axis=X)
        nc.vector.reciprocal(i0[:], gmax[:])
        # e = 2 - gmax*i0
        nc.vector.tensor_scalar(
            out=e[:], in0=gmax[:], scalar1=i0[:, 0:1], scalar2=-1.0,
            op0=Alu.mult, op1=Alu.mult,
        )
        nc.vector.tensor_scalar(
            out=e[:], in0=e[:], scalar1=2.0, scalar2=1.0, op0=Alu.add, op1=Alu.mult,
        )
        # inv = i0*e*255 ; s = gmax/255 ; ms = -MAGIC*s
        nc.vector.tensor_scalar(
            out=scal[:, 0:1], in0=e[:], scalar1=i0[:, 0:1], scalar2=QMAX_U,
            op0=Alu.mult, op1=Alu.mult,
        )
        nc.vector.tensor_scalar(
            out=scal[:, 1:2], in0=gmax[:], scalar1=1.0 / QMAX_U, scalar2=1.0,
            op0=Alu.mult, op1=Alu.mult,
        )
        nc.vector.tensor_scalar(
            out=scal[:, 2:3], in0=scal[:, 1:2], scalar1=-MAGIC, scalar2=1.0,
            op0=Alu.mult, op1=Alu.mult,
        )
        inv_ap = scal[:, 0:1]
        s_ap = scal[:, 1:2]
        ms_ap = scal[:, 2:3]

        tact = ctx.enter_context(tc.tile_pool(name="ta", bufs=2))
        tdve = ctx.enter_context(tc.tile_pool(name="td", bufs=2))
        oact = ctx.enter_context(tc.tile_pool(name="oa", bufs=2))
        odve = ctx.enter_context(tc.tile_pool(name="od", bufs=2))

        # ---- Phase B: out = round(y*inv)*s, RNE via magic constant.
        # 6 chunks run both passes on the Activation engine (~3.8us each), 10
        # both on the DVE (~2.25us each); issued in expected-completion order
        # so the (FIFO) output-DMA queue never head-of-line blocks.
        def chunk_act(c):
            t = tact.tile([P, CH2], f32)
            o = oact.tile([P, CH2], bf16)
            nc.scalar.activation(
                t[:], ybig[:, c * CH2:(c + 1) * CH2], Copy,
                bias=MAGIC, scale=inv_ap,
            )
            # out = t*s - MAGIC*s >= 0 always, so Relu == identity here (and
            # unlike Copy it accepts a per-partition bias AP)
            nc.scalar.activation(o[:], t[:], Relu, bias=ms_ap, scale=s_ap)
            return o

        def chunk_dve(c):
            t = tdve.tile([P, CH2], f32)
            o = odve.tile([P, CH2], bf16)
            nc.vector.tensor_scalar(
                out=t[:], in0=ybig[:, c * CH2:(c + 1) * CH2],
                scalar1=inv_ap, scalar2=MAGIC, op0=Alu.mult, op1=Alu.add,
            )
            nc.vector.tensor_scalar(
                out=o[:], in0=t[:],
                scalar1=-MAGIC, scalar2=s_ap, op0=Alu.add, op1=Alu.mult,
            )
            return o

        def emit_dve_split(c):
            # first DVE chunk as a 512-col quarter then the 1536-col rest:
            # the first output DMA launches after ~0.7us of DVE work instead
            # of ~2.3us, without starving the 2-buffer output pool
            bb, j = divmod(c, NJ)
            for q0, q1 in ((0, 1), (1, MT)):
                cols = (q1 - q0) * NT
                t = tdve.tile([P, cols], f32)
                o = odve.tile([P, cols], bf16)
                sl = slice(c * CH2 + q0 * NT, c * CH2 + q1 * NT)
                nc.vector.tensor_scalar(
                    out=t[:], in0=ybig[:, sl],
                    scalar1=inv_ap, scalar2=MAGIC, op0=Alu.mult, op1=Alu.add,
                )
                nc.vector.tensor_scalar(
                    out=o[:], in0=t[:],
                    scalar1=-MAGIC, scalar2=s_ap, op0=Alu.add, op1=Alu.mult,
                )
                nc.sync.dma_start(
                    out=out[bb, q0 * P:q1 * P, j * NT:(j + 1) * NT].rearrange(
                        "(m p) n -> p m n", p=P
                    ),
                    in_=o[:, :].rearrange("p (m n) -> p m n", m=q1 - q0),
                )

        def emit_dve(c):
            bb, j = divmod(c, NJ)
            o = chunk_dve(c)
            nc.sync.dma_start(
                out=out[bb, :, j * NT:(j + 1) * NT].rearrange(
                    "(m p) n -> p m n", p=P
                ),
                in_=o[:, :].rearrange("p (m n) -> p m n", m=MT),
            )

        def emit_act(c):
            bb, j = divmod(c, NJ)
            o = chunk_act(c)
            nc.sync.dma_start(
                out=out[bb, :, j * NT:(j + 1) * NT].rearrange(
                    "(m p) n -> p m n", p=P
                ),
                in_=o[:, :].rearrange("p (m n) -> p m n", m=MT),
            )

        # expected per-chunk engine times (ns): DVE 2254, Act 3784 (+800 bias
        # so a slightly-late Act chunk never head-of-line blocks the queue)
        acts = [(3784.0 * (i + 1) + 800, c, emit_act) for i, c in enumerate(range(NACT))]
        dves = [
            (2254.0 * (i + 1), c, emit_dve_split if i == 0 else emit_dve)
            for i, c in enumerate(range(NACT, NCH))
        ]
        for _, c, fn in sorted(acts + dves, key=lambda t: t[0]):
            fn(c)
    nc.compile()  # bacc lowering: register allocation, DCE, nop-fusion
    return nc


def _quant_po2(v, qmax):
    # mirrors reference.fake_quant_signed_po2 in float32
    v = np.asarray(v, np.float32)
    qmax = np.float32(qmax)
    maxabs = np.max(np.abs(v)).astype(np.float32)
    ratio = np.float32(maxabs / qmax)
    s = np.exp2(np.ceil(np.log2(ratio))).astype(np.float32)
    return (np.round(np.clip(v / s, -qmax, qmax)).astype(np.float32) * s).astype(
        np.float32
    )


def kernel(x, W, b):
    global LAST_RESULT
    x = np.ascontiguousarray(np.asarray(x, np.float32))
    W = np.asarray(W, np.float32)
    b = np.asarray(b, np.float32)
    assert x.shape == (B, CIN, N) and W.shape == (COUT, CIN) and b.shape == (COUT,)

    Wq = _quant_po2(W, QMAX_S)
    bq = _quant_po2(b, QMAX_S)
    wT_h = np.ascontiguousarray(Wq.T)                      # [CIN, COUT]
    bq_h = np.ascontiguousarray(bq.reshape(MT, P).T)       # [P, MT]

    if "nc" not in _cache:
        _cache["nc"] = _build()
    nc = _cache["nc"]

    in_maps = [
        {"xs": x[c * BSH:(c + 1) * BSH], "wT": wT_h, "bqv": bq_h}
        for c in range(NCORES)
    ]
    res = run_bass_kernel_spmd(nc, in_maps, core_ids=list(range(NCORES)))
    LAST_RESULT = res
    return np.concatenate(
        [np.asarray(res.results[c]["out"]) for c in range(NCORES)], axis=0
    ).astype(np.float32)


if __name__ == "__main__":
    rng = np.random.default_rng(0)
    x = rng.standard_normal((B, CIN, N), np.float32)
    W = (rng.standard_normal((COUT, CIN)) * 0.05).astype(np.float32)
    b = (rng.standard_normal((COUT,)) * 0.1).astype(np.float32)
    y = kernel(x=x, W=W, b=b)
    print("out", y.shape, y.dtype, float(y.min()), float(y.max()))
